# revision 9
# baseline (speedup 1.0000x reference)
"""Trainium2 Bass kernel for nn_DiffusionLayer (ADI diffusion, 10 steps).

Mathematical collapse: every sweep of the ADI scheme is a fixed tridiagonal
solve shared by all rows (the coefficients depend only on the size-128
parameter vectors and the time index, never on u). Each x-sweep is a right
multiplication V <- V @ Mx^T and each y-sweep a left multiplication
V <- My @ V of the 128x128 image V. Left and right multiplications commute,
so the whole 30-sweep scheme is

    V_out = L @ V @ R,   L = My_10 ... My_1,   R = Mx_1^T Mx_2^T ... Mx_20^T

with L, R computed on host in float64 (including the reference's EPS
perturbations of the Thomas recurrences). On device each image needs two
128x128x128 matmuls:

    P1 = matmul(lhsT=V,  rhs=L^T) = (L V)^T      [w  x h']
    W  = matmul(lhsT=P1, rhs=R)   = (L V) R      [h' x w']

Everything on device is bf16 (error gate is 2e-2; measured bf16 error is
~7e-3): halves HBM traffic vs fp32 and runs the PE at 1 cycle/row instead
of 4. DRAM tensors use an [h, b, w] layout so every DMA moves 4KB
contiguous runs per partition (the [b, h, w] layout would give 256B runs,
below the 512B DMA line-rate threshold). Host pre/post-permutes.

PSUM->SBUF drains alternate between the Vector and Scalar engines (either
alone would bottleneck at ~1 elem/cycle from PSUM). The second matmul of
each 4-image quad lags the first by D quads so the intermediate drain is
hidden under PE work.

Sharding: pure data parallelism, 2048 images -> 256 per core across 8 cores.
"""

import numpy as np
import ml_dtypes

import concourse.mybir as mybir
import concourse.tile as tile
from concourse import bacc
from concourse.bass_utils import run_bass_kernel_spmd

N_CORES = 8
BATCH = 2048
S = 128
PER_CORE = BATCH // N_CORES  # 256

SIZE, DT, DX, DY, NUM_STEPS, EPS = 128, 0.01, 1.0, 1.0, 10, 1e-6

GIMG = 16            # images per DMA group (512 KB bf16 per transfer)
NGRP = PER_CORE // GIMG
QUAD = 8             # images per PSUM tile (2 banks) / per drain copy
QPG = GIMG // QUAD   # octs per group
D = 2                # octs of lag between matmul-1 and matmul-2
PREF = 4             # input groups prefetched ahead of compute
OUT_LAG = 1          # groups of output held back so input DMAs lead the wire
TAIL_SPLIT = 2       # last groups whose output DMA is split into halves
WARM_MM = 28         # dummy matmuls to warm the PE clock during the head

BF16 = ml_dtypes.bfloat16


# ----------------------------------------------------------------- host math
def _smooth3(v):
    vp = np.pad(v, (1, 1), mode="edge")
    return (vp[:-2] + vp[1:-1] + vp[2:]) / 3.0


def _thomas_matrix(a, b, c):
    """Matrix M of the reference thomas() linear map d -> x (includes EPS)."""
    n = len(b)
    dn = np.empty(n)
    cs = np.empty(n)
    dn[0] = b[0] + EPS
    cs[0] = c[0] / dn[0]
    for i in range(1, n):
        dn[i] = b[i] - a[i] * cs[i - 1] + EPS
        cs[i] = c[i] / dn[i]
    ds = np.empty((n, n))
    ds[0] = np.eye(n)[0] / dn[0]
    eye = np.eye(n)
    for i in range(1, n):
        ds[i] = (eye[i] - a[i] * ds[i - 1]) / dn[i]
    x = np.empty((n, n))
    x[n - 1] = ds[n - 1]
    for i in range(n - 2, -1, -1):
        x[i] = ds[i] - cs[i] * x[i + 1]
    return x


def _sweep_matrix(vec, dt, dh):
    coeff = _smooth3(vec) * dt / dh**2
    a = -coeff
    c = -coeff
    b = 1.0 + 2.0 * coeff
    b = b.copy()
    b[0] = 1.0 + coeff[0]
    b[-1] = 1.0 + coeff[-1]
    return _thomas_matrix(a, b, c)


def _coef(base, lin, quad, t):
    return np.clip(base + lin * t + quad * t * t, EPS, None)


def _build_lr(abx, atx, aqx, bby, bty, bqy):
    """L (y-operator product) and R (x-operator product) in float64."""
    L = np.eye(SIZE)
    R = np.eye(SIZE)
    t = 0.0
    for _ in range(NUM_STEPS):
        Mx = _sweep_matrix(_coef(abx, atx, aqx, t), DT / 2, DX)
        R = R @ Mx.T
        t += DT / 2
        My = _sweep_matrix(_coef(bby, bty, bqy, t), DT, DY)
        L = My @ L
        t += DT / 2
        Mx = _sweep_matrix(_coef(abx, atx, aqx, t), DT / 2, DX)
        R = R @ Mx.T
    return L, R


# ------------------------------------------------------------- device kernel
_NC_CACHE = {}


def _build_nc():
    if "nc" in _NC_CACHE:
        return _NC_CACHE["nc"]
    bf = mybir.dt.bfloat16
    f32 = mybir.dt.float32
    nc = bacc.Bacc(None)
    # [h, b, w] layout: partition h sees contiguous (b, w) runs
    u_in = nc.dram_tensor("u", [S, PER_CORE, S], bf, kind="ExternalInput")
    lt_in = nc.dram_tensor("lt", [S, S], bf, kind="ExternalInput")
    r_in = nc.dram_tensor("rm", [S, S], bf, kind="ExternalInput")
    u_out = nc.dram_tensor("out", [S, PER_CORE, S], bf, kind="ExternalOutput")

    with tile.TileContext(nc) as tc:
        with (
            tc.tile_pool(name="mats", bufs=1) as mats,
            tc.tile_pool(name="inp", bufs=PREF + 2) as inp,
            tc.tile_pool(name="outp", bufs=OUT_LAG + 4) as outp,
            tc.tile_pool(name="mid", bufs=D + 3) as mid,
            tc.tile_pool(name="ps1", bufs=2, space="PSUM") as ps1,
            tc.tile_pool(name="ps2", bufs=2, space="PSUM") as ps2,
        ):
            lt_s = mats.tile([S, S], bf)
            r_s = mats.tile([S, S], bf)
            warm = mats.tile([S, S], bf)
            nc.sync.dma_start(out=lt_s[:], in_=lt_in[:])
            nc.sync.dma_start(out=r_s[:], in_=r_in[:])

            in_tiles = {}
            ot_tiles = {}
            p1c_tiles = {}
            pending_out = []

            def load_group(g):
                t = inp.tile([S, GIMG, S], bf, name="in_t")
                g0 = g * GIMG
                if g == 0:
                    # fine-grained head loads so the PE starts sooner
                    for c0, cn in ((0, 4), (4, 4), (8, 8)):
                        nc.sync.dma_start(
                            out=t[:, c0 : c0 + cn, :],
                            in_=u_in[:, g0 + c0 : g0 + c0 + cn, :],
                        )
                else:
                    nc.sync.dma_start(out=t[:], in_=u_in[:, g0 : g0 + GIMG, :])
                in_tiles[g] = t

            for g in range(min(PREF, NGRP)):
                load_group(g)

            # Warm the PE clock (HAM un-throttles after ~3.4us of sustained
            # activity) with dummy matmuls on a zeroed tile while the first
            # input DMAs are in flight. Results land in a ps1-pool bank and
            # are overwritten by the first real matmul group.
            nc.vector.memset(warm[:], 0.0)
            wpt = ps1.tile([S, QUAD, S], f32, name="pt1")
            for _ in range(WARM_MM):
                nc.tensor.matmul(wpt[:, 0, :], warm[:], warm[:])

            NQ = NGRP * QPG
            for k in range(NQ + D):
                if k < NQ:
                    g, q = divmod(k, QPG)
                    if q == 0:
                        if g + PREF < NGRP:
                            load_group(g + PREF)
                        ot_tiles[g] = outp.tile([S, GIMG, S], bf, name="ot")
                    it = in_tiles[g]
                    pt = ps1.tile([S, QUAD, S], f32, name="pt1")
                    for j in range(QUAD):
                        nc.tensor.matmul(
                            pt[:, j, :], it[:, q * QUAD + j, :], lt_s[:]
                        )
                    pc = mid.tile([S, QUAD, S], bf, name="p1c")
                    if k % 2 == 0:
                        nc.vector.tensor_copy(out=pc[:], in_=pt[:])
                    else:
                        nc.scalar.copy(out=pc[:], in_=pt[:])
                    p1c_tiles[k] = pc
                    if q == QPG - 1:
                        del in_tiles[g]
                if k >= D:
                    kk = k - D
                    g2, q2 = divmod(kk, QPG)
                    pc = p1c_tiles.pop(kk)
                    pt2 = ps2.tile([S, QUAD, S], f32, name="pt2")
                    for j in range(QUAD):
                        nc.tensor.matmul(pt2[:, j, :], pc[:, j, :], r_s[:])
                    dst = ot_tiles[g2][:, q2 * QUAD : (q2 + 1) * QUAD, :]
                    if kk % 2 == 0:
                        nc.scalar.copy(out=dst, in_=pt2[:])
                    else:
                        nc.vector.tensor_copy(out=dst, in_=pt2[:])
                    if q2 == QPG - 1:
                        pending_out.append(g2)
                        if len(pending_out) > OUT_LAG:
                            og = pending_out.pop(0)
                            og0 = og * GIMG
                            nc.sync.dma_start(
                                out=u_out[:, og0 : og0 + GIMG, :],
                                in_=ot_tiles.pop(og)[:],
                            )
            for og in pending_out:
                og0 = og * GIMG
                ot = ot_tiles.pop(og)
                if og >= NGRP - TAIL_SPLIT:
                    # smaller tail transfers start draining sooner and the
                    # final completion receipt covers fewer bytes
                    half = GIMG // 2
                    for c0 in (0, half):
                        nc.sync.dma_start(
                            out=u_out[:, og0 + c0 : og0 + c0 + half, :],
                            in_=ot[:, c0 : c0 + half, :],
                        )
                else:
                    nc.sync.dma_start(
                        out=u_out[:, og0 : og0 + GIMG, :], in_=ot[:]
                    )

    nc.finalize()
    _NC_CACHE["nc"] = nc
    return nc


# ---------------------------------------------------------------- entrypoint
def _prepare_in_maps(inputs):
    u = np.asarray(inputs["u"], dtype=np.float32)
    assert u.shape == (BATCH, 1, S, S)
    L, R = _build_lr(
        np.asarray(inputs["alpha_base_x"], dtype=np.float64),
        np.asarray(inputs["alpha_time_coeff_x"], dtype=np.float64),
        np.asarray(inputs["alpha_time_quad_x"], dtype=np.float64),
        np.asarray(inputs["beta_base_y"], dtype=np.float64),
        np.asarray(inputs["beta_time_coeff_y"], dtype=np.float64),
        np.asarray(inputs["beta_time_quad_y"], dtype=np.float64),
    )
    lt16 = np.ascontiguousarray(L.T.astype(BF16))
    r16 = np.ascontiguousarray(R.astype(BF16))
    u16 = u[:, 0].astype(BF16)  # (BATCH, S, S)
    in_maps = [
        {
            # [b, h, w] -> [h, b, w]
            "u": np.ascontiguousarray(
                u16[c * PER_CORE : (c + 1) * PER_CORE].transpose(1, 0, 2)
            ),
            "lt": lt16,
            "rm": r16,
        }
        for c in range(N_CORES)
    ]
    return in_maps


def _unpack_out(results):
    # each result "out" is [h, b, w] bf16 -> [b, h, w] fp32
    parts = [
        np.asarray(r["out"]).transpose(1, 0, 2).astype(np.float32)
        for r in results
    ]
    return np.concatenate(parts, axis=0).reshape(BATCH, 1, S, S)


def kernel(**inputs) -> np.ndarray:
    in_maps = _prepare_in_maps(inputs)
    nc = _build_nc()
    res = run_bass_kernel_spmd(nc, in_maps, list(range(N_CORES)))
    return _unpack_out(res.results)


if __name__ == "__main__":
    rng = np.random.default_rng(0)
    fake = {
        "u": rng.standard_normal((BATCH, 1, S, S), dtype=np.float32),
        "alpha_base_x": np.full(S, 2.0, np.float32),
        "alpha_base_y": np.full(S, 2.0, np.float32),
        "beta_base_x": np.full(S, 2.0, np.float32),
        "beta_base_y": np.full(S, 2.0, np.float32),
        "alpha_time_coeff_x": 0.01 * rng.standard_normal(S).astype(np.float32),
        "alpha_time_coeff_y": 0.01 * rng.standard_normal(S).astype(np.float32),
        "beta_time_coeff_x": 0.01 * rng.standard_normal(S).astype(np.float32),
        "beta_time_coeff_y": 0.01 * rng.standard_normal(S).astype(np.float32),
        "alpha_time_quad_x": 0.01 * rng.standard_normal(S).astype(np.float32),
        "alpha_time_quad_y": 0.01 * rng.standard_normal(S).astype(np.float32),
        "beta_time_quad_x": 0.01 * rng.standard_normal(S).astype(np.float32),
        "beta_time_quad_y": 0.01 * rng.standard_normal(S).astype(np.float32),
    }
    out = kernel(**fake)
    print("kernel output:", out.shape, out.dtype)
